# revision 1
# baseline (speedup 1.0000x reference)
"""Single-head attention Trainium2 kernel (batch=8 data-parallel over 8 cores).

Reference computation (per batch element b):
    Q = x @ Wq; K = x @ Wk; V = x @ Wv          (x: [S, D], W*: [D, O])
    out = softmax(Q @ K.T * SCALE) @ V          (SCALE = 1/8, hardcoded sqrt(64))

Kernel strategy (per core, one batch element):

  scores are reassociated:  Q K^T = x (Wq Wk^T) x^T = x M x^T
  output is reassociated:   attn V = (attn x) Wv
      M   = Wq Wk^T                       (1024^3, bf16)
      T^T[d',s] = sum_d M[d,d'] xT[d,s]
      S^T[k,q]  = sum_d' xT[d',k] T^T[d',q]
      A^T[d,q]  = sum_s x[s,d] exp(S^T)[s,q]
      out[q,o]  = sum_d A^T[d,q] Wv[d,o]  * 1/rowsum

  The three big matmul groups (T^T, S^T, A^T) run as hi/lo-compensated
  fp8 DoubleRow: each operand is split X = X_hi + X_lo (both fp8), and
  A B ~= A_hi B_hi + A_lo B_hi + A_hi B_lo — 3 DoubleRow matmuls per
  2 contraction chunks = 0.75x the PE cycles of bf16, at ~bf16 accuracy
  (absmax rel err ~7e-3 vs the fp32 reference). M is scaled by 64 into
  e4m3's normal range; the fused exp scale compensates (1/8/64).
  x / T^T / M pairs use e4m3; exp weights / x-for-A^T use e5m2 (range).
  The out projection stays bf16 (A^T is unnormalized, too large for fp8).

  Schedule: w loads lead (they gate Wq/Wk transposes -> M), x follows,
  cast on the Pool engine. T^T is produced s-block-major and interleaved
  with phase C q-blocks (qb n only needs T^T columns n*256..n*256+255),
  keeping it off the critical path. All operands stay resident in SBUF.
"""

import sys

sys.path.insert(0, "/opt/trn_rl_repo")

from contextlib import ExitStack

import numpy as np

import concourse.bass as bass
import concourse.mybir as mybir
from concourse import bacc
from concourse.tile import TileContext
from concourse.masks import make_identity

F32 = mybir.dt.float32
B16 = mybir.dt.bfloat16
E4 = mybir.dt.float8e4
E5 = mybir.dt.float8e5
EXP = mybir.ActivationFunctionType.Exp
COPY = mybir.ActivationFunctionType.Copy
DR = mybir.MatmulPerfMode.DoubleRow
MUL = mybir.AluOpType.mult
SUB = mybir.AluOpType.subtract
MSCALE = 64.0                     # M is stored as 64*M (e4m3 normal range)
SCALE = 1.0 / 8.0 / MSCALE        # fused into exp; 1/sqrt(64) from the module


def build_attn(S=2048, D=1024, O=1024, QB=256, reps=1):
    """Build the Bass module for one core: x[S,D], w[3,D,O] -> out[S,O]."""
    DC = D // 128   # 8 chunks of d
    OC = O // 128   # 8 chunks of o
    KC = S // 128   # 16 chunks of s (keys)
    NQB = S // QB   # 8 q-blocks
    QC = QB // 128  # 2
    NG = KC // 4    # x-transpose groups of 4 kc (512 cols of xt)

    nc = bacc.Bacc("TRN2", target_bir_lowering=False, debug=False)
    x_in = nc.dram_tensor("x", [S, D], F32, kind="ExternalInput")
    w_in = nc.dram_tensor("w", [3, D, O], F32, kind="ExternalInput")
    out_d = nc.dram_tensor("out", [S, O], F32, kind="ExternalOutput")

    def pair(t, two=2):
        return t.rearrange("p (two n) -> p two n", two=two)

    with TileContext(nc) as tc:
      for _rep in range(reps):
        top = ExitStack()
        const_pool = top.enter_context(tc.tile_pool(name="constp", bufs=1))
        xn2_pool = top.enter_context(tc.tile_pool(name="xn2p", bufs=KC))
        xt_pool = top.enter_context(tc.tile_pool(name="xtp", bufs=DC))
        wv_pool = top.enter_context(tc.tile_pool(name="wvp", bufs=DC))
        m_pool = top.enter_context(tc.tile_pool(name="mp", bufs=DC))

        ident_f = const_pool.tile([128, 128], F32, tag="identf")
        make_identity(nc, ident_f)
        ident = const_pool.tile([128, 128], B16, tag="identb")
        nc.vector.tensor_copy(out=ident, in_=ident_f)
        ones_f = const_pool.tile([128, 1], F32, tag="onesf")
        nc.gpsimd.memset(ones_f, 1.0 / 256.0)
        ones_e5 = const_pool.tile([128, 1], E5, tag="onese5")
        nc.vector.tensor_copy(out=ones_e5, in_=ones_f)

        xn2 = [None] * KC            # x natural   [128 s, D] bf16 (transposes)
        xt_h = [None] * (DC // 2)    # x^T hi/lo   [128 d, 2, S] e4m3 pairs
        xt_l = [None] * (DC // 2)
        m_h = [None] * (DC // 2)     # 64*M hi/lo  [128 d, 2, D] e4m3 pairs
        m_l = [None] * (DC // 2)
        wv_h = [None] * (DC // 2)    # Wv hi/lo [128 d, 2, O] e5m2 pairs
        wv_l = [None] * (DC // 2)

        # ---------------- Phase AB ----------------
        with ExitStack() as ab:
            xf_pool = ab.enter_context(tc.tile_pool(name="xfp", bufs=3))
            wf_pool = ab.enter_context(tc.tile_pool(name="wfp", bufs=3))
            wn_pool = ab.enter_context(tc.tile_pool(name="wnp", bufs=10))
            wt_pool = ab.enter_context(tc.tile_pool(name="wtp", bufs=2 * OC))
            psT = ab.enter_context(tc.tile_pool(name="psT", bufs=3, space="PSUM"))
            psW = ab.enter_context(tc.tile_pool(name="psW", bufs=3, space="PSUM"))
            psM = ab.enter_context(tc.tile_pool(name="psM", bufs=2, space="PSUM"))

            wn = {0: [None] * DC, 1: [None] * DC}

            def load_w(j):
                for dc in range(DC):
                    wf = wf_pool.tile([128, O], F32, tag="wf", bufs=3)
                    nc.sync.dma_start(out=wf, in_=w_in[j, dc * 128:(dc + 1) * 128, :])
                    wn[j][dc] = wn_pool.tile(
                        [128, O], B16, tag="wn", bufs=10, name=f"wn{j}_{dc}"
                    )
                    if dc % 2 == 0:
                        nc.vector.tensor_copy(out=wn[j][dc], in_=wf)
                    else:
                        nc.scalar.copy(out=wn[j][dc], in_=wf)

            def transpose_w(j):
                for oc in range(OC):
                    ps = psW.tile([128, D], B16, tag="psW", bufs=2)
                    for dc in range(DC):
                        nc.tensor.transpose(
                            ps[:, dc * 128:(dc + 1) * 128],
                            wn[j][dc][:, oc * 128:(oc + 1) * 128],
                            ident,
                        )
                    wt[j][oc] = wt_pool.tile(
                        [128, D], B16, tag="wt", bufs=2 * OC, name=f"wt{j}_{oc}"
                    )
                    if oc % 2 == 0:
                        nc.vector.tensor_copy(out=wt[j][oc], in_=ps)
                    else:
                        nc.scalar.copy(out=wt[j][oc], in_=ps)

            wt = {0: [None] * OC, 1: [None] * OC}
            # w0 loads+casts; w1 DMAs early (casts deferred past wq evicts)
            load_w(0)
            w1f = []
            for dc in range(DC):
                wf = wf_pool.tile([128, O], F32, tag="w1f", bufs=DC)
                nc.sync.dma_start(out=wf, in_=w_in[1, dc * 128:(dc + 1) * 128, :])
                w1f.append(wf)
            # x DMAs (sync, behind w) + Pool casts to resident bf16
            for kc in range(KC):
                xf = xf_pool.tile([128, D], F32, tag="xf", bufs=3)
                nc.sync.dma_start(out=xf, in_=x_in[kc * 128:(kc + 1) * 128, :])
                xn2[kc] = xn2_pool.tile(
                    [128, D], B16, tag="xn2", bufs=KC, name=f"xn2_{kc}"
                )
                nc.gpsimd.tensor_copy(out=xn2[kc], in_=xf)
            # wq transposes chase w0 casts
            transpose_w(0)
            # w1 casts (data has landed), then wk transposes
            for dc in range(DC):
                wn[1][dc] = wn_pool.tile(
                    [128, O], B16, tag="wn", bufs=10, name=f"wn1_{dc}"
                )
                if dc % 2 == 0:
                    nc.vector.tensor_copy(out=wn[1][dc], in_=w1f[dc])
                else:
                    nc.scalar.copy(out=wn[1][dc], in_=w1f[dc])
            transpose_w(1)

            # M[d, d'] (PE-bound) interleaved with x-transpose groups
            # (eviction-bound): xt hi/lo evictions run during M's PE time.
            def emit_m(dt):
                if dt % 2 == 0:
                    m_h[dt // 2] = m_pool.tile(
                        [128, 2 * D], E4, tag="mh", bufs=DC // 2, name=f"mh_{dt//2}")
                    m_l[dt // 2] = m_pool.tile(
                        [128, 2 * D], E4, tag="ml", bufs=DC // 2, name=f"ml_{dt//2}")
                for half in range(2):
                    ps = psM.tile([128, 512], F32, tag="psM", bufs=2)
                    for oc in range(OC):
                        nc.tensor.matmul(
                            ps,
                            wt[0][oc][:, dt * 128:(dt + 1) * 128],
                            wt[1][oc][:, half * 512:(half + 1) * 512],
                            start=(oc == 0), stop=(oc == OC - 1),
                        )
                    c0 = (dt % 2) * D + half * 512
                    hi = m_h[dt // 2][:, c0:c0 + 512]
                    nc.scalar.activation(out=hi, in_=ps, func=COPY, scale=MSCALE)
                    nc.vector.scalar_tensor_tensor(
                        out=m_l[dt // 2][:, c0:c0 + 512],
                        in0=ps, scalar=MSCALE, in1=hi, op0=MUL, op1=SUB)

            def emit_xg(gp, dc):
                if gp == 0 and dc % 2 == 0:
                    xt_h[dc // 2] = xt_pool.tile(
                        [128, 2 * S], E4, tag="xth", bufs=DC // 2,
                        name=f"xth_{dc//2}")
                    xt_l[dc // 2] = xt_pool.tile(
                        [128, 2 * S], E4, tag="xtl", bufs=DC // 2,
                        name=f"xtl_{dc//2}")
                ps = psT.tile([128, 1024], B16, tag="psT", bufs=3)
                for i in range(8):
                    kc = 8 * gp + i
                    nc.tensor.transpose(
                        ps[:, i * 128:(i + 1) * 128],
                        xn2[kc][:, dc * 128:(dc + 1) * 128],
                        ident,
                    )
                c0 = (dc % 2) * S + gp * 1024
                hi = xt_h[dc // 2][:, c0:c0 + 1024]
                nc.scalar.copy(out=hi, in_=ps)
                nc.vector.scalar_tensor_tensor(
                    out=xt_l[dc // 2][:, c0:c0 + 1024],
                    in0=ps, scalar=1.0, in1=hi, op0=MUL, op1=SUB)

            for dt in range(DC):
                emit_m(dt)
                if dt % 2 == 1:
                    base = (dt // 2) * 4
                    for u in range(4):
                        emit_xg((base + u) // DC, (base + u) % DC)

            # wv loads + e5m2 hi/lo pair casts; needed only in phase C
            for dc in range(DC):
                wvf = wf_pool.tile([128, O], F32, tag="wf", bufs=3)
                nc.sync.dma_start(out=wvf, in_=w_in[2, dc * 128:(dc + 1) * 128, :])
                dp, sl = dc // 2, dc % 2
                if sl == 0:
                    wv_h[dp] = wv_pool.tile(
                        [128, 2 * O], E5, tag="wvh", bufs=DC // 2, name=f"wvh_{dp}")
                    wv_l[dp] = wv_pool.tile(
                        [128, 2 * O], E5, tag="wvl", bufs=DC // 2, name=f"wvl_{dp}")
                hi = wv_h[dp][:, sl * O:(sl + 1) * O]
                nc.scalar.copy(out=hi, in_=wvf)
                lo = wv_l[dp][:, sl * O:(sl + 1) * O]
                if dc % 2 == 0:
                    nc.vector.scalar_tensor_tensor(
                        out=lo, in0=wvf, scalar=1.0, in1=hi, op0=MUL, op1=SUB)
                else:
                    nc.gpsimd.tensor_sub(out=lo, in0=wvf, in1=hi)


        # ------------- post-AB pools (reuse freed staging space) -------------
        tt_pool = top.enter_context(tc.tile_pool(name="ttp", bufs=DC))
        xnp_pool = top.enter_context(tc.tile_pool(name="xnpp", bufs=KC))
        tt_h = [None] * (DC // 2)    # T^T hi/lo [128 d', 2, S] e4m3 pairs
        tt_l = [None] * (DC // 2)
        xn_h = [None] * (KC // 2)    # x hi/lo [128 s, 2, D] e5m2 pairs (A^T)
        xn_l = [None] * (KC // 2)

        with ExitStack() as ph_c:
            eb_pool = ph_c.enter_context(tc.tile_pool(name="ebp", bufs=4))
            e_pool = ph_c.enter_context(tc.tile_pool(name="ep", bufs=KC + 6))
            at_pool = ph_c.enter_context(tc.tile_pool(name="atp", bufs=DC // 2 + 2))
            outs_pool = ph_c.enter_context(tc.tile_pool(name="outsp", bufs=4))
            small_pool = ph_c.enter_context(tc.tile_pool(name="smallp", bufs=4 * QC))
            pcs = ph_c.enter_context(tc.tile_pool(name="pcs", bufs=3, space="PSUM"))
            pcsum = ph_c.enter_context(tc.tile_pool(name="pcsum", bufs=1, space="PSUM"))
            pca = ph_c.enter_context(tc.tile_pool(name="pca", bufs=2, space="PSUM"))
            pco = ph_c.enter_context(tc.tile_pool(name="pco", bufs=2, space="PSUM"))

            def dr3(dst, lhs_hl, rhs_hl, np_, first, last):
                """3-term compensated DR accumulation over np_ pairs."""
                i = 0
                n = 3 * np_
                for pp in range(np_):
                    lh, ll = lhs_hl(pp)
                    rh, rl = rhs_hl(pp)
                    for (a, b) in ((lh, rh), (ll, rh), (lh, rl)):
                        nc.tensor.matmul(
                            dst, a, b,
                            start=(first and i == 0), stop=(last and i == n - 1),
                            perf_mode=DR)
                        i += 1

            def emit_tt_sb(sbh):
                """T^T for s-block sbh (512 cols = 2 DR q-blocks of 256)."""
                for pc in range(DC):
                    if sbh == 0 and pc % 2 == 0:
                        tt_h[pc // 2] = tt_pool.tile(
                            [128, 2 * S], E4, tag="tth", bufs=DC // 2,
                            name=f"tth_{pc//2}")
                        tt_l[pc // 2] = tt_pool.tile(
                            [128, 2 * S], E4, tag="ttl", bufs=DC // 2,
                            name=f"ttl_{pc//2}")
                    ps = pcs.tile([128, 512], F32, tag="pcs", bufs=3)
                    for h2 in range(2):
                        sbq = 2 * sbh + h2
                        dst = ps[:, h2 * 256:(h2 + 1) * 256]
                        dr3(dst,
                            lambda dp: (
                                pair(m_h[dp])[:, :, pc * 128:(pc + 1) * 128],
                                pair(m_l[dp])[:, :, pc * 128:(pc + 1) * 128]),
                            lambda dp: (
                                pair(xt_h[dp])[:, :, sbq * 256:(sbq + 1) * 256],
                                pair(xt_l[dp])[:, :, sbq * 256:(sbq + 1) * 256]),
                            DC // 2, True, True)
                    c0 = (pc % 2) * S + sbh * 512
                    hi = tt_h[pc // 2][:, c0:c0 + 512]
                    nc.scalar.copy(out=hi, in_=ps)
                    nc.vector.scalar_tensor_tensor(
                        out=tt_l[pc // 2][:, c0:c0 + 512],
                        in0=ps, scalar=1.0, in1=hi, op0=MUL, op1=SUB)

            def emit_qb(qb):
                q0 = qb * QB
                # scoresT[ks, q] via DR; one wide exp per kc-pair -> bf16,
                # then e5m2 hi/lo split (hi on DVE, lo on Pool).
                e_his, e_los = [], []
                for kp in range(KC // 2):
                    ps_s = pcs.tile([128, 2 * QB], F32, tag="pcs", bufs=3)
                    for half in range(2):
                        kc = 2 * kp + half
                        dst = ps_s[:, half * QB:(half + 1) * QB]
                        dr3(dst,
                            lambda pp: (
                                pair(xt_h[pp])[:, :, kc * 128:(kc + 1) * 128],
                                pair(xt_l[pp])[:, :, kc * 128:(kc + 1) * 128]),
                            lambda pp: (
                                pair(tt_h[pp])[:, :, q0:q0 + QB],
                                pair(tt_l[pp])[:, :, q0:q0 + QB]),
                            DC // 2, True, True)
                    eb = eb_pool.tile([128, 2 * QB], B16, tag="eb", bufs=4)
                    nc.scalar.activation(out=eb, in_=ps_s, func=EXP, scale=SCALE)
                    eh = e_pool.tile([128, 2 * QB], E5, tag="eh", bufs=KC // 2 + 3)
                    nc.vector.tensor_copy(out=eh, in_=eb)
                    el = e_pool.tile([128, 2 * QB], E5, tag="el", bufs=KC // 2 + 3)
                    nc.gpsimd.tensor_sub(out=el, in0=eb, in1=eh)
                    e_his.append(eh)
                    e_los.append(el)
                # A^T[d, q] via DR; two dc per PSUM bank. Evicted as e5m2
                # hi/lo pairs scaled by 2^-8 (the ones-const folds the 2^8
                # back into the reciprocals).
                aT_h, aT_l = [], []
                for dp in range(DC // 2):
                    ps_a = pca.tile([128, 2 * QB], F32, tag="pca", bufs=2)
                    for half in range(2):
                        dc = 2 * dp + half
                        dst = ps_a[:, half * QB:(half + 1) * QB]
                        dr3(dst,
                            lambda kp: (
                                pair(xn_h[kp])[:, :, dc * 128:(dc + 1) * 128],
                                pair(xn_l[kp])[:, :, dc * 128:(dc + 1) * 128]),
                            lambda kp: (pair(e_his[kp]), pair(e_los[kp])),
                            KC // 2, True, True)
                    a_h = at_pool.tile([128, 2 * QB], E5, tag="aTh", bufs=DC // 2 + 2)
                    nc.vector.tensor_scalar_mul(out=a_h, in0=ps_a, scalar1=1.0 / 256.0)
                    a_l = at_pool.tile([128, 2 * QB], E5, tag="aTl", bufs=DC // 2 + 2)
                    nc.vector.scalar_tensor_tensor(
                        out=a_l, in0=ps_a, scalar=1.0 / 256.0, in1=a_h,
                        op0=MUL, op1=SUB)
                    aT_h.append(a_h)
                    aT_l.append(a_l)
                # row sums over ks via ones-matmuls on e_hi + e_lo (fp8,
                # normal mode, N=1), then reciprocal.
                recips = []
                for qc in range(QC):
                    ps_sum = pcsum.tile([128, 1], F32, tag="pcsum", bufs=1)
                    n = 2 * KC
                    i = 0
                    for kp in range(KC // 2):
                        for half in range(2):
                            sl = half * QB + qc * 128
                            for e in (e_his[kp], e_los[kp]):
                                nc.tensor.matmul(
                                    ps_sum, e[:, sl:sl + 128], ones_e5,
                                    start=(i == 0), stop=(i == n - 1))
                                i += 1
                    rc = small_pool.tile([128, 1], F32, tag="recip", bufs=4 * QC)
                    nc.vector.reciprocal(out=rc, in_=ps_sum)
                    recips.append(rc)
                # out[q, o] = A @ Wv via DR, normalized on eviction
                for qc in range(QC):
                    for ohh in range(O // 512):
                        ps_o = pco.tile([128, 512], F32, tag="pco", bufs=2)
                        for half in range(2):
                            oh = 2 * ohh + half
                            dst = ps_o[:, half * 256:(half + 1) * 256]
                            dr3(dst,
                                lambda dp: (
                                    pair(aT_h[dp])[:, :, qc * 128:(qc + 1) * 128],
                                    pair(aT_l[dp])[:, :, qc * 128:(qc + 1) * 128]),
                                lambda dp: (
                                    pair(wv_h[dp])[:, :, oh * 256:(oh + 1) * 256],
                                    pair(wv_l[dp])[:, :, oh * 256:(oh + 1) * 256]),
                                DC // 2, True, True)
                        os_ = outs_pool.tile([128, 512], F32, tag="outs", bufs=4)
                        nc.vector.tensor_scalar_mul(
                            out=os_, in0=ps_o, scalar1=recips[qc])
                        nc.sync.dma_start(
                            out=out_d[
                                q0 + qc * 128:q0 + (qc + 1) * 128,
                                ohh * 512:(ohh + 1) * 512,
                            ],
                            in_=os_,
                        )

            def derive_xn():
                # x e5m2 hi/lo pairs for A^T, from resident bf16 x; emitted
                # after tt-sb0 so tt evictions aren't queued behind them.
                for kp in range(KC // 2):
                    xn_h[kp] = xnp_pool.tile(
                        [128, 2 * D], E5, tag="xnh", bufs=KC // 2, name=f"xnh_{kp}")
                    xn_l[kp] = xnp_pool.tile(
                        [128, 2 * D], E5, tag="xnl", bufs=KC // 2, name=f"xnl_{kp}")
                    for sl in range(2):
                        kc = 2 * kp + sl
                        hi = xn_h[kp][:, sl * D:(sl + 1) * D]
                        lo = xn_l[kp][:, sl * D:(sl + 1) * D]
                        if kc % 2 == 0:
                            nc.vector.tensor_copy(out=hi, in_=xn2[kc])
                            nc.gpsimd.tensor_sub(out=lo, in0=xn2[kc], in1=hi)
                        else:
                            nc.scalar.copy(out=hi, in_=xn2[kc])
                            nc.vector.scalar_tensor_tensor(
                                out=lo, in0=xn2[kc], scalar=1.0, in1=hi,
                                op0=MUL, op1=SUB)

            # qb n reads tt[:, n*QB:(n+1)*QB]; s-block g covers qb 2g, 2g+1.
            for g in range(S // 512):
                emit_tt_sb(g)
                if g == 0:
                    derive_xn()
                emit_qb(2 * g)
                emit_qb(2 * g + 1)

        top.close()

    nc.compile()
    return nc


_NC_CACHE = {}


def _get_nc():
    key = "full"
    if key not in _NC_CACHE:
        _NC_CACHE[key] = build_attn()
    return _NC_CACHE[key]


def kernel(**inputs):
    """Full-input entry point: x [8, 2048, 1024], kernel [3, 1024, 1024]."""
    from concourse.bass_utils import run_bass_kernel_spmd

    x = np.ascontiguousarray(inputs["x"], dtype=np.float32)
    w = np.ascontiguousarray(inputs["kernel"], dtype=np.float32)
    B = x.shape[0]
    nc = _get_nc()
    in_maps = [{"x": x[b], "w": w} for b in range(B)]
    res = run_bass_kernel_spmd(nc, in_maps, core_ids=list(range(B)))
    return np.stack([res.results[b]["out"] for b in range(B)], axis=0)

